# revision 15
# baseline (speedup 1.0000x reference)
"""TRN2 Bass kernel for ConvNeXt-MLP + parallel top-2-of-3 LoRA-MoE.

Data-parallel over the token dim across 8 NeuronCores (12544 tokens ->
1568/core). All weights replicated. Per core, everything runs in
feature-major ("transposed") layout: activations live in SBUF as
[features_on_partitions, tokens_on_free_dim]; the host transposes x in and
the output back out, and pre-tiles x/w1/w2/consts into the exact SBUF
layouts so every DMA is a contiguous block.

Per core (T = 1568 tokens, 4 token tiles of 392):
  base:   outT = w2^T @ gelu(w1^T @ xT + b1) + b2          (f32r matmuls)
  router: merged [rw|wd]^T @ xT in f32r (1 cy/row). Logits go token-major
          via DVE stream-transposes (32x32 blocks) + partition-shifted
          copies, overlapped tile-by-tile under the main matmul stream.
          Softmax + top-2-of-3 + renormalize run as ~11 batched DVE ops
          over all 13 token chunks at once (stride-0 broadcast APs); the
          router bias is folded in as exp(rb) since softmax is shift/scale
          invariant. The Exp (which swaps the ACT table away from Gelu) is
          emitted at the g0->g1 boundary inside a w2-chain window where the
          ACT engine is idle. comb goes back to expert-major via 13 PE
          transposes interleaved one-at-a-time into g1's dense matmul
          stream (keeps the PE p-state hot).
  lora:   actT = gelu(wd^T @ xT); scaled = actT * expand(comb), expand
          matmuls interleaved into g2; moeT = wu^T @ scaled accumulated
          into the same PSUM banks as the base output in g3.

Hidden dim (3072 = 24 chunks) is processed in 4 groups of 6 chunks so that
w1/w2 stream through SBUF exactly once (prefetched one group ahead on the
scalar/gpsimd DMA queues; xt + output stores ride the sync queue). The
j-loop runs a depth-3 software pipeline (h j0..j2 before the first w2
chain) so the next tile's PSUM reuse never waits on this tile's accumulate
copies.
"""

import numpy as np

import concourse.bacc as bacc
import concourse.mybir as mybir
import concourse.tile as tile
from concourse.bass_utils import run_bass_kernel_spmd

F32 = mybir.dt.float32
F32R = mybir.dt.float32r
BF16 = mybir.dt.bfloat16
AF = mybir.ActivationFunctionType
ALU = mybir.AluOpType
AX = mybir.AxisListType

NCORES = 8
B, N, D = 64, 196, 768
T = B * N                  # 12544 tokens total
TC = T // NCORES           # 1568 tokens per core
HID = 4 * D                # 3072
E, R = 3, 8
ER = E * R                 # 24
DC = D // 128              # 6 input-feature chunks
HC = HID // 128            # 24 hidden chunks
MC = D // 128              # 6 output chunks
NGROUPS = 4
GH = HC // NGROUPS         # 6 hidden chunks per group
NT = 4
TN = TC // NT              # 392 tokens per tile
NRC = 13                   # router 128-token chunks (12x128 + 32)
RC_N = [128] * 12 + [32]
CHUNKS_BY_TILE = [[0, 1, 2], [3, 4, 5], [6, 7, 8], [9, 10, 11, 12]]

# const blob column offsets: f32r blobs (PE-consumed) and f32 blob
RWD0 = 0
CWR = 336
BX0, WU0, ID0 = 0, 24, 792
CWL = ID0 + 128
B10, B20, ERB0 = 0, 24, 30
CWF = 33

_cache = {}


def _build():
    nc = bacc.Bacc("TRN2", target_bir_lowering=False, debug=False)

    xt_d = nc.dram_tensor("xt", [NT * 128, DC * TN], F32R,
                          kind="ExternalInput")
    w1_d = nc.dram_tensor("w1", [NGROUPS * 128, GH * DC * 128], F32R,
                          kind="ExternalInput")
    w2_d = nc.dram_tensor("w2", [NGROUPS * 128, GH * D], F32R,
                          kind="ExternalInput")
    cbr_d = nc.dram_tensor("cblobr", [128, CWR], F32R, kind="ExternalInput")
    cbl_d = nc.dram_tensor("cblobl", [128, CWL], F32R, kind="ExternalInput")
    cbf_d = nc.dram_tensor("cblobf", [128, CWF], F32, kind="ExternalInput")
    out_d = nc.dram_tensor("outT", [NT * 128, MC * TN], BF16,
                           kind="ExternalOutput")

    with tile.TileContext(nc) as tc:
        with (
            tc.tile_pool(name="const", bufs=1) as cp,
            tc.tile_pool(name="big", bufs=1) as bp,
            tc.tile_pool(name="wts", bufs=2) as wp,
            tc.tile_pool(name="hbuf", bufs=5) as hp,
        ):
            # ---- resident loads ----
            cbr = cp.tile([128, CWR], F32R, tag="cbr")
            nc.gpsimd.dma_start(cbr[:], cbr_d[:])
            cbl = cp.tile([128, CWL], F32R, tag="cbl")
            cbf = cp.tile([128, CWF], F32, tag="cbf")
            nc.gpsimd.dma_start(cbf[:], cbf_d[:])
            rwd = cbr[:, RWD0:RWD0 + DC * 56]
            bx = cbl[0:E, BX0:BX0 + ER]
            wu = cbl[0:ER, WU0:WU0 + D]
            ident = cbl[:, ID0:ID0 + 128]
            b1 = cbf[:, B10:B10 + HC]
            b2 = cbf[:, B20:B20 + MC]
            erb = cbf[:, ERB0:ERB0 + E]
            # warm the gelu ACT table before the DMA wait so the first real
            # gelu doesn't eat a 1.3us table load
            warm = cp.tile([128, 1], F32, tag="warm")
            nc.scalar.activation(warm[:], warm[:], AF.Gelu)

            def load_xt(i):
                x_i = bp.tile([128, DC * TN], F32R, tag=f"xt{i}",
                              name=f"xt{i}")
                half = DC * TN // 2
                for k in range(2):
                    nc.sync.dma_start(
                        x_i[:, k * half:(k + 1) * half],
                        xt_d[i * 128:(i + 1) * 128, k * half:(k + 1) * half])
                return x_i
            xts = [load_xt(i) for i in range(NT)]

            lgT = bp.tile([E, TC], F32, tag="lgT")
            acts = bp.tile([ER, TC], F32, tag="acts")
            comb_t = bp.tile([E, TC], F32R, tag="combt")
            scaled = bp.tile([ER, TC], F32R, tag="scaled")
            acc = bp.tile([128, MC * TC], F32, tag="acc")
            stg = bp.tile([128, NRC * 32], F32, tag="stg")
            ttok = bp.tile([128, NRC * 32], F32, tag="ttok")
            ctok = bp.tile([128, NRC * 3], F32R, tag="ctok")
            prb = bp.tile([128, NRC * 3], F32, tag="prb")
            ssum = bp.tile([128, NRC], F32, tag="ssum")
            pmin = bp.tile([128, NRC], F32, tag="pmin")
            rs = bp.tile([128, NRC], F32, tag="rs")
            den = bp.tile([128, NRC], F32, tag="den")
            invd = bp.tile([128, NRC], F32, tag="invd")
            t1 = bp.tile([128, NRC], F32, tag="t1")
            msk = bp.tile([128, NRC * 3], F32, tag="msk")
            obs = [bp.tile([128, MC * TN], BF16, tag=f"ob{i}",
                           name=f"ob{i}") for i in range(2)]

            def stage_logit_chunks(cis):
                """lgT [3,TC] -> token-major ttok staging, on DVE only."""
                for ci in cis:
                    nblk = RC_N[ci] // 32
                    for k in range(nblk):
                        lo = 128 * ci + 32 * k
                        nc.vector.tensor_copy(
                            stg[32 * k:32 * k + 3, 32 * ci:32 * ci + 32],
                            lgT[0:3, lo:lo + 32],
                        )
                    nc.vector.transpose(
                        ttok[:32 * nblk, 32 * ci:32 * ci + 32],
                        stg[:32 * nblk, 32 * ci:32 * ci + 32],
                    )

            def softmax_comb():
                """Batched softmax + top-2-of-3 + renormalize over all
                chunks; writes token-major comb into ctok."""
                t3 = ttok[:].rearrange("p (c x) -> p c x", c=NRC)[:, :, 0:3]
                p3 = prb[:].rearrange("p (c e) -> p c e", c=NRC)
                nc.scalar.activation(p3, t3, AF.Exp)
                e3 = erb.unsqueeze(1).broadcast_to([128, NRC, 3])
                nc.vector.tensor_tensor(p3, p3, e3, op=ALU.mult)
                nc.vector.tensor_reduce(ssum[:], p3, axis=AX.X, op=ALU.add)
                nc.vector.tensor_reduce(pmin[:], p3, axis=AX.X, op=ALU.min)
                nc.vector.reciprocal(rs[:], ssum[:])
                nc.vector.tensor_sub(den[:], ssum[:], pmin[:])
                nc.vector.tensor_mul(den[:], den[:], rs[:])
                nc.vector.tensor_scalar_add(den[:], den[:], 1e-6)
                nc.vector.reciprocal(invd[:], den[:])
                nc.vector.tensor_mul(t1[:], rs[:], invd[:])
                m3 = msk[:].rearrange("p (c e) -> p c e", c=NRC)
                pm3 = pmin[:].unsqueeze(2).broadcast_to([128, NRC, 3])
                nc.vector.tensor_tensor(m3, p3, pm3, op=ALU.is_gt)
                nc.vector.tensor_tensor(m3, m3, p3, op=ALU.mult)
                c3 = ctok[:].rearrange("p (c e) -> p c e", c=NRC)
                t13 = t1[:].unsqueeze(2).broadcast_to([128, NRC, 3])
                nc.vector.tensor_tensor(c3, m3, t13, op=ALU.mult)

            def load_w1g(g, nch=2):
                w1g = wp.tile([128, GH * DC * 128], F32R, tag="w1g",
                              name=f"w1g_{g}")
                step = GH * DC * 128 // nch
                for k in range(nch):
                    nc.scalar.dma_start(
                        w1g[:, k * step:(k + 1) * step],
                        w1_d[g * 128:(g + 1) * 128, k * step:(k + 1) * step])
                return w1g

            def load_w2g(g, nch=2, bsplit=False):
                w2g = wp.tile([128, GH * D], F32R, tag="w2g",
                              name=f"w2g_{g}")
                step = GH * D // nch
                for k in range(nch):
                    eng = nc.gpsimd
                    eng.dma_start(
                        w2g[:, k * step:(k + 1) * step],
                        w2_d[g * 128:(g + 1) * 128, k * step:(k + 1) * step])
                return w2g

            # deferred PE-side tasks, interleaved one per j-iteration into
            # the dense matmul stream so the PE array never cools down
            side_pe = []

            def emit_comb_transpose(ci, psH):
                n = RC_N[ci]
                tp = psH.tile([128, 512], F32, tag="h", name=f"tp_{ci}")
                nc.tensor.transpose(tp[:E, :n].bitcast(F32R),
                                    ctok[0:n, 3 * ci:3 * ci + 3],
                                    ident[:n, :n])
                nc.scalar.copy(comb_t[:, 128 * ci:128 * ci + n], tp[:E, :n])

            def emit_expand(i, psH):
                t0 = i * TN
                ex = psH.tile([128, 512], F32, tag="h", name=f"ex_{i}")
                nc.tensor.matmul(ex[:ER, :TN], bx, comb_t[:, t0:t0 + TN],
                                 start=True, stop=True)
                nc.vector.tensor_mul(scaled[:, t0:t0 + TN],
                                     acts[:, t0:t0 + TN], ex[:ER, :TN])

            # ---- main stream: 4 groups of 6 hidden chunks ----
            with (
                tc.tile_pool(name="psO", bufs=1, space="PSUM") as psO,
                tc.tile_pool(name="psH", bufs=2, space="PSUM") as psH,
            ):
                w1q = [load_w1g(0)]
                w2q = [load_w2g(0, nch=GH)]
                # late-needed consts (wu/ident/bx) after the g0 weight stream
                nc.gpsimd.dma_start(cbl[:], cbl_d[:])
                w1q.append(load_w1g(1))
                w2q.append(load_w2g(1))
                for g in range(NGROUPS):
                    if g >= 1 and g + 1 < NGROUPS:
                        w1q.append(load_w1g(g + 1))
                        w2q.append(load_w2g(g + 1))
                    w1g, w2g = w1q[g], w2q[g]

                    if g == 1:
                        # ACT is idle during g0-t3's w2 chains: do the Exp
                        # (and its two table swaps) there, then queue the 13
                        # comb transposes for interleaving into g1
                        softmax_comb()
                        side_pe.extend(
                            (lambda ci=ci: emit_comb_transpose(ci, psH))
                            for ci in range(NRC))
                    if g == 2:
                        side_pe.extend(
                            (lambda i=i: emit_expand(i, psH))
                            for i in range(NT))

                    t0 = 0
                    for nt in range(NT):
                        n = TN
                        if g == 0:
                            # merged router + LoRA-down matmul, this tile
                            dn27 = psH.tile([128, 512], F32, tag="h",
                                            name=f"dn27_{nt}")
                            for c in range(DC):
                                nc.tensor.matmul(
                                    dn27[:56, :n],
                                    rwd[:, c * 56:(c + 1) * 56],
                                    xts[nt][:, c * n:(c + 1) * n],
                                    start=(c == 0), stop=(c == DC - 1),
                                )
                            nc.vector.tensor_copy(lgT[:, t0:t0 + n],
                                                  dn27[:E, :n])
                            nc.scalar.activation(acts[:, t0:t0 + n],
                                                 dn27[32:56, :n], AF.Gelu)
                            stage_logit_chunks(CHUNKS_BY_TILE[nt])

                        outp = [psO.tile([128, 512], F32, tag=f"out{m}",
                                         name=f"out{m}_{g}_{nt}")
                                for m in range(MC)]
                        hsb = [None] * GH
                        for j in range(GH + 3):
                            if j < GH:
                                hps = psH.tile([128, 512], F32, tag="h",
                                               name=f"h_{g}_{nt}_{j}")
                                for c in range(DC):
                                    nc.tensor.matmul(
                                        hps[:, :n],
                                        w1g[:, (j * DC + c) * 128:
                                               (j * DC + c) * 128 + 128],
                                        xts[nt][:, c * n:(c + 1) * n],
                                        start=(c == 0), stop=(c == DC - 1),
                                    )
                                hsb[j] = hp.tile([128, 512], F32R, tag="hs",
                                                 name=f"hs_{g}_{nt}_{j}")
                                nc.scalar.activation(
                                    hsb[j][:, :n], hps[:, :n], AF.Gelu,
                                    bias=b1[:, g * GH + j:g * GH + j + 1],
                                )
                                if side_pe and j >= 2:
                                    side_pe.pop(0)()
                            if j >= 3:
                                jj = j - 3
                                for m in range(MC):
                                    nc.tensor.matmul(
                                        outp[m][:, :n],
                                        w2g[:, jj * D + m * 128:
                                               jj * D + m * 128 + 128],
                                        hsb[jj][:, :n],
                                        start=(jj == 0),
                                        stop=(jj == GH - 1 and g < NGROUPS - 1),
                                    )
                        if g == NGROUPS - 1:
                            for m in range(MC):
                                nc.tensor.matmul(
                                    outp[m][:, :n],
                                    wu[:, m * 128:(m + 1) * 128],
                                    scaled[:, t0:t0 + n],
                                    start=False, stop=True,
                                )
                        ob = obs[nt % 2]
                        for m in range(MC):
                            a = acc[:, m * TC + t0:m * TC + t0 + n]
                            if g == 0:
                                nc.vector.tensor_copy(a, outp[m][:, :n])
                            elif g < NGROUPS - 1:
                                nc.vector.tensor_add(a, a, outp[m][:, :n])
                            else:
                                nc.vector.scalar_tensor_tensor(
                                    ob[:, m * n:(m + 1) * n],
                                    outp[m][:, :n], b2[:, m:m + 1], a,
                                    op0=ALU.add, op1=ALU.add,
                                )
                                if m in (2, MC - 1):
                                    lo = 0 if m == 2 else 3 * n
                                    nc.sync.dma_start(
                                        out_d[nt * 128:(nt + 1) * 128,
                                              lo:(m + 1) * n],
                                        ob[:, lo:(m + 1) * n],
                                    )
                        t0 += n

    nc.compile()
    return nc


def _pack_consts(b1, b2, router_w, router_b, w_down, w_up):
    cbr = np.zeros((128, CWR), np.float32)
    rwd = np.zeros((DC, 128, 56), np.float32)
    rw = np.asarray(router_w, np.float32).reshape(DC, 128, E)
    wd = np.asarray(w_down, np.float32).transpose(1, 0, 2).reshape(DC, 128, ER)
    rwd[:, :, :E] = rw
    rwd[:, :, 32:] = wd
    cbr[:, RWD0:RWD0 + DC * 56] = rwd.transpose(1, 0, 2).reshape(128, DC * 56)
    cbl = np.zeros((128, CWL), np.float32)
    cbl[0:E, BX0:BX0 + ER] = np.repeat(np.eye(E, dtype=np.float32), R, axis=1)
    cbl[0:ER, WU0:WU0 + D] = np.asarray(w_up, np.float32).reshape(ER, D)
    cbl[:, ID0:ID0 + 128] = np.eye(128, dtype=np.float32)
    cbf = np.zeros((128, CWF), np.float32)
    cbf[:, B10:B10 + HC] = np.asarray(b1, np.float32).reshape(HC, 128).T
    cbf[:, B20:B20 + MC] = np.asarray(b2, np.float32).reshape(MC, 128).T
    cbf[:, ERB0:ERB0 + E] = np.exp(np.asarray(router_b, np.float32))[None, :]
    return cbr, cbl, cbf


def _prep_inputs(x, w1, b1, w2, b2, router_w, router_b, w_down, w_up):
    x = np.ascontiguousarray(np.asarray(x, dtype=np.float32))
    xT = x.reshape(T, D).T  # [D, T]
    # w1 [D, HID] -> [g, p, j, c, f128] -> [(g p), j*c*128]
    w1p = np.asarray(w1, np.float32).reshape(DC, 128, NGROUPS, GH, 128)
    w1p = np.ascontiguousarray(w1p.transpose(2, 1, 3, 0, 4)).reshape(
        NGROUPS * 128, GH * DC * 128)
    # w2 [HID, D] -> [g, p, j, dout] -> [(g p), j*D]
    w2p = np.asarray(w2, np.float32).reshape(NGROUPS, GH, 128, D)
    w2p = np.ascontiguousarray(w2p.transpose(0, 2, 1, 3)).reshape(
        NGROUPS * 128, GH * D)
    cbr, cbl, cbf = _pack_consts(b1, b2, router_w, router_b, w_down, w_up)
    common = {
        "w1": w1p,
        "w2": w2p,
        "cblobr": cbr,
        "cblobl": cbl,
        "cblobf": cbf,
    }
    in_maps = []
    for c in range(NCORES):
        m = dict(common)
        xc = xT[:, c * TC:(c + 1) * TC].reshape(DC, 128, NT, TN)
        m["xt"] = np.ascontiguousarray(xc.transpose(2, 1, 0, 3)).reshape(
            NT * 128, DC * TN)
        in_maps.append(m)
    return in_maps


def _run(inputs, trace=False):
    if "nc" not in _cache:
        _cache["nc"] = _build()
    nc = _cache["nc"]
    in_maps = _prep_inputs(**inputs)
    res = run_bass_kernel_spmd(nc, in_maps, core_ids=list(range(NCORES)),
                               trace=trace)
    outs = []
    for c in range(NCORES):
        a = np.asarray(res.results[c]["outT"]).astype(np.float32)
        a = a.reshape(NT, 128, MC, TN)
        outs.append(a.transpose(2, 1, 0, 3).reshape(D, TC))
    outT = np.concatenate(outs, axis=1)  # [D, T]
    out = np.ascontiguousarray(outT.T).reshape(B, N, D).astype(np.float32)
    return out, res


def kernel(**inputs):
    return _run(inputs)[0]


# revision 16
# speedup vs baseline: 1.0154x; 1.0154x over previous
"""TRN2 Bass kernel for ConvNeXt-MLP + parallel top-2-of-3 LoRA-MoE.

Data-parallel over the token dim across 8 NeuronCores (12544 tokens ->
1568/core). All weights replicated. Per core, everything runs in
feature-major ("transposed") layout: activations live in SBUF as
[features_on_partitions, tokens_on_free_dim]; the host transposes x in and
the output back out, and pre-tiles x/w1/w2/consts into the exact SBUF
layouts so every DMA is a contiguous block.

Per core (T = 1568 tokens, 4 token tiles of 392):
  base:   outT = w2^T @ gelu(w1^T @ xT + b1) + b2          (f32r matmuls)
  router: merged [rw|wd]^T @ xT in f32r (1 cy/row). Logits go token-major
          via DVE stream-transposes (32x32 blocks) + partition-shifted
          copies, overlapped tile-by-tile under the main matmul stream.
          Softmax + top-2-of-3 + renormalize run as ~11 batched DVE ops
          over all 13 token chunks at once (stride-0 broadcast APs); the
          router bias is folded in as exp(rb) since softmax is shift/scale
          invariant. The Exp (which swaps the ACT table away from Gelu) is
          emitted at the g0->g1 boundary inside a w2-chain window where the
          ACT engine is idle. comb goes back to expert-major via 13 PE
          transposes interleaved one-at-a-time into g1's dense matmul
          stream (keeps the PE p-state hot).
  lora:   actT = gelu(wd^T @ xT); scaled = actT * expand(comb), expand
          matmuls interleaved into g2; moeT = wu^T @ scaled accumulated
          into the same PSUM banks as the base output in g3.

Hidden dim (3072 = 24 chunks) is processed in 4 groups of 6 chunks so that
w1/w2 stream through SBUF exactly once (prefetched one group ahead on the
scalar/gpsimd DMA queues; xt + output stores ride the sync queue). The
j-loop runs a depth-3 software pipeline (h j0..j2 before the first w2
chain) so the next tile's PSUM reuse never waits on this tile's accumulate
copies.
"""

import numpy as np

import concourse.bacc as bacc
import concourse.mybir as mybir
import concourse.tile as tile
from concourse.bass_utils import run_bass_kernel_spmd

F32 = mybir.dt.float32
F32R = mybir.dt.float32r
BF16 = mybir.dt.bfloat16
AF = mybir.ActivationFunctionType
ALU = mybir.AluOpType
AX = mybir.AxisListType

NCORES = 8
B, N, D = 64, 196, 768
T = B * N                  # 12544 tokens total
TC = T // NCORES           # 1568 tokens per core
HID = 4 * D                # 3072
E, R = 3, 8
ER = E * R                 # 24
DC = D // 128              # 6 input-feature chunks
HC = HID // 128            # 24 hidden chunks
MC = D // 128              # 6 output chunks
NGROUPS = 4
GH = HC // NGROUPS         # 6 hidden chunks per group
NT = 4
TN = TC // NT              # 392 tokens per tile
NRC = 13                   # router 128-token chunks (12x128 + 32)
RC_N = [128] * 12 + [32]
CHUNKS_BY_TILE = [[0, 1, 2], [3, 4, 5], [6, 7, 8], [9, 10, 11, 12]]

# const blob column offsets: f32r blobs (PE-consumed) and f32 blob
RWD0 = 0
CWR = 336
BX0, WU0, ID0 = 0, 24, 792
CWL = ID0 + 128
B10, B20, ERB0 = 0, 24, 30
CWF = 33

_cache = {}


def _build():
    nc = bacc.Bacc("TRN2", target_bir_lowering=False, debug=False)

    xt_d = nc.dram_tensor("xt", [NT * 128, DC * TN], F32R,
                          kind="ExternalInput")
    w1_d = nc.dram_tensor("w1", [NGROUPS * 128, GH * DC * 128], F32R,
                          kind="ExternalInput")
    w2_d = nc.dram_tensor("w2", [NGROUPS * 128, GH * D], F32R,
                          kind="ExternalInput")
    cbr_d = nc.dram_tensor("cblobr", [128, CWR], F32R, kind="ExternalInput")
    cbl_d = nc.dram_tensor("cblobl", [128, CWL], F32R, kind="ExternalInput")
    cbf_d = nc.dram_tensor("cblobf", [128, CWF], F32, kind="ExternalInput")
    out_d = nc.dram_tensor("outT", [NT * 128, MC * TN], BF16,
                           kind="ExternalOutput")

    with tile.TileContext(nc) as tc:
        with (
            tc.tile_pool(name="const", bufs=1) as cp,
            tc.tile_pool(name="big", bufs=1) as bp,
            tc.tile_pool(name="wts", bufs=2) as wp,
            tc.tile_pool(name="hbuf", bufs=5) as hp,
        ):
            # ---- resident loads ----
            cbr = cp.tile([128, CWR], F32R, tag="cbr")
            nc.gpsimd.dma_start(cbr[:], cbr_d[:])
            cbl = cp.tile([128, CWL], F32R, tag="cbl")
            cbf = cp.tile([128, CWF], F32, tag="cbf")
            nc.scalar.dma_start(cbf[:], cbf_d[:])
            rwd = cbr[:, RWD0:RWD0 + DC * 56]
            bx = cbl[0:E, BX0:BX0 + ER]
            wu = cbl[0:ER, WU0:WU0 + D]
            ident = cbl[:, ID0:ID0 + 128]
            b1 = cbf[:, B10:B10 + HC]
            b2 = cbf[:, B20:B20 + MC]
            erb = cbf[:, ERB0:ERB0 + E]
            # warm the gelu ACT table before the DMA wait so the first real
            # gelu doesn't eat a 1.3us table load
            warm = cp.tile([128, 1], F32, tag="warm")
            nc.scalar.activation(warm[:], warm[:], AF.Gelu)

            def load_xt(i):
                x_i = bp.tile([128, DC * TN], F32R, tag=f"xt{i}",
                              name=f"xt{i}")
                nch = 1 if i == 0 else 2
                step = DC * TN // nch
                for k in range(nch):
                    nc.sync.dma_start(
                        x_i[:, k * step:(k + 1) * step],
                        xt_d[i * 128:(i + 1) * 128, k * step:(k + 1) * step])
                return x_i
            xts = [load_xt(i) for i in range(NT)]

            lgT = bp.tile([E, TC], F32, tag="lgT")
            acts = bp.tile([ER, TC], F32, tag="acts")
            comb_t = bp.tile([E, TC], F32R, tag="combt")
            scaled = bp.tile([ER, TC], F32R, tag="scaled")
            acc = bp.tile([128, MC * TC], F32, tag="acc")
            stg = bp.tile([128, NRC * 32], F32, tag="stg")
            ttok = bp.tile([128, NRC * 32], F32, tag="ttok")
            ctok = bp.tile([128, NRC * 3], F32R, tag="ctok")
            prb = bp.tile([128, NRC * 3], F32, tag="prb")
            ssum = bp.tile([128, NRC], F32, tag="ssum")
            pmin = bp.tile([128, NRC], F32, tag="pmin")
            rs = bp.tile([128, NRC], F32, tag="rs")
            den = bp.tile([128, NRC], F32, tag="den")
            invd = bp.tile([128, NRC], F32, tag="invd")
            t1 = bp.tile([128, NRC], F32, tag="t1")
            msk = bp.tile([128, NRC * 3], F32, tag="msk")
            obs = [bp.tile([128, MC * TN], BF16, tag=f"ob{i}",
                           name=f"ob{i}") for i in range(2)]

            def stage_logit_chunks(cis):
                """lgT [3,TC] -> token-major ttok staging, on DVE only."""
                for ci in cis:
                    nblk = RC_N[ci] // 32
                    for k in range(nblk):
                        lo = 128 * ci + 32 * k
                        nc.vector.tensor_copy(
                            stg[32 * k:32 * k + 3, 32 * ci:32 * ci + 32],
                            lgT[0:3, lo:lo + 32],
                        )
                    nc.vector.transpose(
                        ttok[:32 * nblk, 32 * ci:32 * ci + 32],
                        stg[:32 * nblk, 32 * ci:32 * ci + 32],
                    )

            def softmax_comb():
                """Batched softmax + top-2-of-3 + renormalize over all
                chunks; writes token-major comb into ctok."""
                t3 = ttok[:].rearrange("p (c x) -> p c x", c=NRC)[:, :, 0:3]
                p3 = prb[:].rearrange("p (c e) -> p c e", c=NRC)
                nc.scalar.activation(p3, t3, AF.Exp)
                e3 = erb.unsqueeze(1).broadcast_to([128, NRC, 3])
                nc.vector.tensor_tensor(p3, p3, e3, op=ALU.mult)
                nc.vector.tensor_reduce(ssum[:], p3, axis=AX.X, op=ALU.add)
                nc.vector.tensor_reduce(pmin[:], p3, axis=AX.X, op=ALU.min)
                nc.vector.reciprocal(rs[:], ssum[:])
                nc.vector.tensor_sub(den[:], ssum[:], pmin[:])
                nc.vector.tensor_mul(den[:], den[:], rs[:])
                nc.vector.tensor_scalar_add(den[:], den[:], 1e-6)
                nc.vector.reciprocal(invd[:], den[:])
                nc.vector.tensor_mul(t1[:], rs[:], invd[:])
                m3 = msk[:].rearrange("p (c e) -> p c e", c=NRC)
                pm3 = pmin[:].unsqueeze(2).broadcast_to([128, NRC, 3])
                nc.vector.tensor_tensor(m3, p3, pm3, op=ALU.is_gt)
                nc.vector.tensor_tensor(m3, m3, p3, op=ALU.mult)
                c3 = ctok[:].rearrange("p (c e) -> p c e", c=NRC)
                t13 = t1[:].unsqueeze(2).broadcast_to([128, NRC, 3])
                nc.vector.tensor_tensor(c3, m3, t13, op=ALU.mult)

            def load_w1g(g, nch=2):
                w1g = wp.tile([128, GH * DC * 128], F32R, tag="w1g",
                              name=f"w1g_{g}")
                step = GH * DC * 128 // nch
                for k in range(nch):
                    nc.scalar.dma_start(
                        w1g[:, k * step:(k + 1) * step],
                        w1_d[g * 128:(g + 1) * 128, k * step:(k + 1) * step])
                return w1g

            def load_w2g(g, nch=2, bsplit=False):
                w2g = wp.tile([128, GH * D], F32R, tag="w2g",
                              name=f"w2g_{g}")
                step = GH * D // nch
                for k in range(nch):
                    eng = nc.gpsimd
                    eng.dma_start(
                        w2g[:, k * step:(k + 1) * step],
                        w2_d[g * 128:(g + 1) * 128, k * step:(k + 1) * step])
                return w2g

            # deferred PE-side tasks, interleaved one per j-iteration into
            # the dense matmul stream so the PE array never cools down
            side_pe = []

            def emit_comb_transpose(ci, psH):
                n = RC_N[ci]
                tp = psH.tile([128, 512], F32, tag="h", name=f"tp_{ci}")
                nc.tensor.transpose(tp[:E, :n].bitcast(F32R),
                                    ctok[0:n, 3 * ci:3 * ci + 3],
                                    ident[:n, :n])
                nc.scalar.copy(comb_t[:, 128 * ci:128 * ci + n], tp[:E, :n])

            def emit_expand(i, psH):
                t0 = i * TN
                ex = psH.tile([128, 512], F32, tag="h", name=f"ex_{i}")
                nc.tensor.matmul(ex[:ER, :TN], bx, comb_t[:, t0:t0 + TN],
                                 start=True, stop=True)
                nc.vector.tensor_mul(scaled[:, t0:t0 + TN],
                                     acts[:, t0:t0 + TN], ex[:ER, :TN])

            # ---- main stream: 4 groups of 6 hidden chunks ----
            with (
                tc.tile_pool(name="psO", bufs=1, space="PSUM") as psO,
                tc.tile_pool(name="psH", bufs=2, space="PSUM") as psH,
            ):
                w1q = [load_w1g(0)]
                w2q = [load_w2g(0)]
                # late-needed consts (wu/ident/bx) after the g0 weight stream
                nc.gpsimd.dma_start(cbl[:], cbl_d[:])
                w1q.append(load_w1g(1))
                w2q.append(load_w2g(1))
                for g in range(NGROUPS):
                    if g >= 1 and g + 1 < NGROUPS:
                        w1q.append(load_w1g(g + 1))
                        w2q.append(load_w2g(g + 1))
                    w1g, w2g = w1q[g], w2q[g]

                    if g == 1:
                        # ACT is idle during g0-t3's w2 chains: do the Exp
                        # (and its two table swaps) there, then queue the 13
                        # comb transposes for interleaving into g1
                        softmax_comb()
                        side_pe.extend(
                            (lambda ci=ci: emit_comb_transpose(ci, psH))
                            for ci in range(NRC))
                    if g == 2:
                        side_pe.extend(
                            (lambda i=i: emit_expand(i, psH))
                            for i in range(NT))

                    t0 = 0
                    for nt in range(NT):
                        n = TN
                        if g == 0:
                            # merged router + LoRA-down matmul, this tile
                            dn27 = psH.tile([128, 512], F32, tag="h",
                                            name=f"dn27_{nt}")
                            for c in range(DC):
                                nc.tensor.matmul(
                                    dn27[:56, :n],
                                    rwd[:, c * 56:(c + 1) * 56],
                                    xts[nt][:, c * n:(c + 1) * n],
                                    start=(c == 0), stop=(c == DC - 1),
                                )
                            nc.vector.tensor_copy(lgT[:, t0:t0 + n],
                                                  dn27[:E, :n])
                            nc.scalar.activation(acts[:, t0:t0 + n],
                                                 dn27[32:56, :n], AF.Gelu)
                            stage_logit_chunks(CHUNKS_BY_TILE[nt])

                        outp = [psO.tile([128, 512], F32, tag=f"out{m}",
                                         name=f"out{m}_{g}_{nt}")
                                for m in range(MC)]
                        hsb = [None] * GH
                        for j in range(GH + 3):
                            if j < GH:
                                hps = psH.tile([128, 512], F32, tag="h",
                                               name=f"h_{g}_{nt}_{j}")
                                for c in range(DC):
                                    nc.tensor.matmul(
                                        hps[:, :n],
                                        w1g[:, (j * DC + c) * 128:
                                               (j * DC + c) * 128 + 128],
                                        xts[nt][:, c * n:(c + 1) * n],
                                        start=(c == 0), stop=(c == DC - 1),
                                    )
                                hsb[j] = hp.tile([128, 512], F32R, tag="hs",
                                                 name=f"hs_{g}_{nt}_{j}")
                                nc.scalar.activation(
                                    hsb[j][:, :n], hps[:, :n], AF.Gelu,
                                    bias=b1[:, g * GH + j:g * GH + j + 1],
                                )
                                if side_pe and j >= 2:
                                    side_pe.pop(0)()
                            if j >= 3:
                                jj = j - 3
                                for m in range(MC):
                                    nc.tensor.matmul(
                                        outp[m][:, :n],
                                        w2g[:, jj * D + m * 128:
                                               jj * D + m * 128 + 128],
                                        hsb[jj][:, :n],
                                        start=(jj == 0),
                                        stop=(jj == GH - 1 and g < NGROUPS - 1),
                                    )
                        if g == NGROUPS - 1:
                            for m in range(MC):
                                nc.tensor.matmul(
                                    outp[m][:, :n],
                                    wu[:, m * 128:(m + 1) * 128],
                                    scaled[:, t0:t0 + n],
                                    start=False, stop=True,
                                )
                        ob = obs[nt % 2]
                        for m in range(MC):
                            a = acc[:, m * TC + t0:m * TC + t0 + n]
                            if g == 0:
                                nc.vector.tensor_copy(a, outp[m][:, :n])
                            elif g < NGROUPS - 1:
                                nc.vector.tensor_add(a, a, outp[m][:, :n])
                            else:
                                nc.vector.scalar_tensor_tensor(
                                    ob[:, m * n:(m + 1) * n],
                                    outp[m][:, :n], b2[:, m:m + 1], a,
                                    op0=ALU.add, op1=ALU.add,
                                )
                                if m in (2, MC - 1):
                                    lo = 0 if m == 2 else 3 * n
                                    nc.sync.dma_start(
                                        out_d[nt * 128:(nt + 1) * 128,
                                              lo:(m + 1) * n],
                                        ob[:, lo:(m + 1) * n],
                                    )
                        t0 += n

    nc.compile()
    return nc


def _pack_consts(b1, b2, router_w, router_b, w_down, w_up):
    cbr = np.zeros((128, CWR), np.float32)
    rwd = np.zeros((DC, 128, 56), np.float32)
    rw = np.asarray(router_w, np.float32).reshape(DC, 128, E)
    wd = np.asarray(w_down, np.float32).transpose(1, 0, 2).reshape(DC, 128, ER)
    rwd[:, :, :E] = rw
    rwd[:, :, 32:] = wd
    cbr[:, RWD0:RWD0 + DC * 56] = rwd.transpose(1, 0, 2).reshape(128, DC * 56)
    cbl = np.zeros((128, CWL), np.float32)
    cbl[0:E, BX0:BX0 + ER] = np.repeat(np.eye(E, dtype=np.float32), R, axis=1)
    cbl[0:ER, WU0:WU0 + D] = np.asarray(w_up, np.float32).reshape(ER, D)
    cbl[:, ID0:ID0 + 128] = np.eye(128, dtype=np.float32)
    cbf = np.zeros((128, CWF), np.float32)
    cbf[:, B10:B10 + HC] = np.asarray(b1, np.float32).reshape(HC, 128).T
    cbf[:, B20:B20 + MC] = np.asarray(b2, np.float32).reshape(MC, 128).T
    cbf[:, ERB0:ERB0 + E] = np.exp(np.asarray(router_b, np.float32))[None, :]
    return cbr, cbl, cbf


def _prep_inputs(x, w1, b1, w2, b2, router_w, router_b, w_down, w_up):
    x = np.ascontiguousarray(np.asarray(x, dtype=np.float32))
    xT = x.reshape(T, D).T  # [D, T]
    # w1 [D, HID] -> [g, p, j, c, f128] -> [(g p), j*c*128]
    w1p = np.asarray(w1, np.float32).reshape(DC, 128, NGROUPS, GH, 128)
    w1p = np.ascontiguousarray(w1p.transpose(2, 1, 3, 0, 4)).reshape(
        NGROUPS * 128, GH * DC * 128)
    # w2 [HID, D] -> [g, p, j, dout] -> [(g p), j*D]
    w2p = np.asarray(w2, np.float32).reshape(NGROUPS, GH, 128, D)
    w2p = np.ascontiguousarray(w2p.transpose(0, 2, 1, 3)).reshape(
        NGROUPS * 128, GH * D)
    cbr, cbl, cbf = _pack_consts(b1, b2, router_w, router_b, w_down, w_up)
    common = {
        "w1": w1p,
        "w2": w2p,
        "cblobr": cbr,
        "cblobl": cbl,
        "cblobf": cbf,
    }
    in_maps = []
    for c in range(NCORES):
        m = dict(common)
        xc = xT[:, c * TC:(c + 1) * TC].reshape(DC, 128, NT, TN)
        m["xt"] = np.ascontiguousarray(xc.transpose(2, 1, 0, 3)).reshape(
            NT * 128, DC * TN)
        in_maps.append(m)
    return in_maps


def _run(inputs, trace=False):
    if "nc" not in _cache:
        _cache["nc"] = _build()
    nc = _cache["nc"]
    in_maps = _prep_inputs(**inputs)
    res = run_bass_kernel_spmd(nc, in_maps, core_ids=list(range(NCORES)),
                               trace=trace)
    outs = []
    for c in range(NCORES):
        a = np.asarray(res.results[c]["outT"]).astype(np.float32)
        a = a.reshape(NT, 128, MC, TN)
        outs.append(a.transpose(2, 1, 0, 3).reshape(D, TC))
    outT = np.concatenate(outs, axis=1)  # [D, T]
    out = np.ascontiguousarray(outT.T).reshape(B, N, D).astype(np.float32)
    return out, res


def kernel(**inputs):
    return _run(inputs)[0]


# revision 17
# speedup vs baseline: 1.0217x; 1.0062x over previous
"""TRN2 Bass kernel for ConvNeXt-MLP + parallel top-2-of-3 LoRA-MoE.

Data-parallel over the token dim across 8 NeuronCores (12544 tokens ->
1568/core). All weights replicated. Per core, everything runs in
feature-major ("transposed") layout: activations live in SBUF as
[features_on_partitions, tokens_on_free_dim]; the host transposes x in and
the output back out, and pre-tiles x/w1/w2/consts into the exact SBUF
layouts so every DMA is a contiguous block.

Per core (T = 1568 tokens, 4 token tiles of 392):
  base:   outT = w2^T @ gelu(w1^T @ xT + b1) + b2          (f32r matmuls)
  router: merged [rw|wd]^T @ xT in f32r (1 cy/row). Logits go token-major
          via DVE stream-transposes (32x32 blocks) + partition-shifted
          copies, overlapped tile-by-tile under the main matmul stream.
          Softmax + top-2-of-3 + renormalize run as ~11 batched DVE ops
          over all 13 token chunks at once (stride-0 broadcast APs); the
          router bias is folded in as exp(rb) since softmax is shift/scale
          invariant. The Exp (which swaps the ACT table away from Gelu) is
          emitted at the g0->g1 boundary inside a w2-chain window where the
          ACT engine is idle. comb goes back to expert-major via 13 PE
          transposes interleaved one-at-a-time into g1's dense matmul
          stream (keeps the PE p-state hot).
  lora:   actT = gelu(wd^T @ xT); scaled = actT * expand(comb), expand
          matmuls interleaved into g2; moeT = wu^T @ scaled accumulated
          into the same PSUM banks as the base output in g3.

Hidden dim (3072 = 24 chunks) is processed in 4 groups of 6 chunks so that
w1/w2 stream through SBUF exactly once (prefetched one group ahead on the
scalar/gpsimd DMA queues; xt + output stores ride the sync queue). The
j-loop runs a depth-3 software pipeline (h j0..j2 before the first w2
chain) so the next tile's PSUM reuse never waits on this tile's accumulate
copies.
"""

import numpy as np

import concourse.bacc as bacc
import concourse.mybir as mybir
import concourse.tile as tile
from concourse.bass_utils import run_bass_kernel_spmd

F32 = mybir.dt.float32
F32R = mybir.dt.float32r
BF16 = mybir.dt.bfloat16
AF = mybir.ActivationFunctionType
ALU = mybir.AluOpType
AX = mybir.AxisListType

NCORES = 8
B, N, D = 64, 196, 768
T = B * N                  # 12544 tokens total
TC = T // NCORES           # 1568 tokens per core
HID = 4 * D                # 3072
E, R = 3, 8
ER = E * R                 # 24
DC = D // 128              # 6 input-feature chunks
HC = HID // 128            # 24 hidden chunks
MC = D // 128              # 6 output chunks
NGROUPS = 4
GH = HC // NGROUPS         # 6 hidden chunks per group
NT = 4
TN = TC // NT              # 392 tokens per tile
NRC = 13                   # router 128-token chunks (12x128 + 32)
RC_N = [128] * 12 + [32]
CHUNKS_BY_TILE = [[0, 1, 2], [3, 4, 5], [6, 7, 8], [9, 10, 11, 12]]

# const blob column offsets: f32r blobs (PE-consumed) and f32 blob
RWD0 = 0
CWR = 336
BX0, WU0, ID0 = 0, 24, 792
CWL = ID0 + 128
B10, B20, ERB0 = 0, 24, 30
CWF = 33

_cache = {}


def _build():
    nc = bacc.Bacc("TRN2", target_bir_lowering=False, debug=False)

    xt_d = nc.dram_tensor("xt", [NT * 128, DC * TN], F32R,
                          kind="ExternalInput")
    w1_d = nc.dram_tensor("w1", [NGROUPS * 128, GH * DC * 128], F32R,
                          kind="ExternalInput")
    w2_d = nc.dram_tensor("w2", [NGROUPS * 128, GH * D], F32R,
                          kind="ExternalInput")
    cbr_d = nc.dram_tensor("cblobr", [128, CWR], F32R, kind="ExternalInput")
    cbl_d = nc.dram_tensor("cblobl", [128, CWL], F32R, kind="ExternalInput")
    cbf_d = nc.dram_tensor("cblobf", [128, CWF], F32, kind="ExternalInput")
    out_d = nc.dram_tensor("outT", [NT * 128, MC * TN], BF16,
                           kind="ExternalOutput")

    with tile.TileContext(nc) as tc:
        with (
            tc.tile_pool(name="const", bufs=1) as cp,
            tc.tile_pool(name="big", bufs=1) as bp,
            tc.tile_pool(name="wts", bufs=2) as wp,
            tc.tile_pool(name="hbuf", bufs=5) as hp,
        ):
            # ---- resident loads ----
            cbr = cp.tile([128, CWR], F32R, tag="cbr")
            nc.gpsimd.dma_start(cbr[:], cbr_d[:])
            cbl = cp.tile([128, CWL], F32R, tag="cbl")
            cbf = cp.tile([128, CWF], F32, tag="cbf")
            nc.scalar.dma_start(cbf[:], cbf_d[:])
            rwd = cbr[:, RWD0:RWD0 + DC * 56]
            bx = cbl[0:E, BX0:BX0 + ER]
            wu = cbl[0:ER, WU0:WU0 + D]
            ident = cbl[:, ID0:ID0 + 128]
            b1 = cbf[:, B10:B10 + HC]
            b2 = cbf[:, B20:B20 + MC]
            erb = cbf[:, ERB0:ERB0 + E]
            # warm the gelu ACT table before the DMA wait so the first real
            # gelu doesn't eat a 1.3us table load
            warm = cp.tile([128, 1], F32, tag="warm")
            nc.scalar.activation(warm[:], warm[:], AF.Gelu)

            def load_xt(i):
                x_i = bp.tile([128, DC * TN], F32R, tag=f"xt{i}",
                              name=f"xt{i}")
                nch = 1 if i == 0 else 2
                step = DC * TN // nch
                for k in range(nch):
                    nc.sync.dma_start(
                        x_i[:, k * step:(k + 1) * step],
                        xt_d[i * 128:(i + 1) * 128, k * step:(k + 1) * step])
                return x_i
            xts = [load_xt(i) for i in range(NT)]

            lgT = bp.tile([E, TC], F32, tag="lgT")
            acts = bp.tile([ER, TC], F32, tag="acts")
            comb_t = bp.tile([E, TC], F32R, tag="combt")
            scaled = bp.tile([ER, TC], F32R, tag="scaled")
            acc = bp.tile([128, MC * TC], F32, tag="acc")
            stg = bp.tile([128, NRC * 32], F32, tag="stg")
            ttok = bp.tile([128, NRC * 32], F32, tag="ttok")
            ctok = bp.tile([128, NRC * 3], F32R, tag="ctok")
            prb = bp.tile([128, NRC * 3], F32, tag="prb")
            ssum = bp.tile([128, NRC], F32, tag="ssum")
            pmin = bp.tile([128, NRC], F32, tag="pmin")
            rs = bp.tile([128, NRC], F32, tag="rs")
            den = bp.tile([128, NRC], F32, tag="den")
            invd = bp.tile([128, NRC], F32, tag="invd")
            t1 = bp.tile([128, NRC], F32, tag="t1")
            msk = bp.tile([128, NRC * 3], F32, tag="msk")
            obs = [bp.tile([128, MC * TN], BF16, tag=f"ob{i}",
                           name=f"ob{i}") for i in range(2)]

            def stage_logit_chunks(cis):
                """lgT [3,TC] -> token-major ttok staging, on DVE only."""
                for ci in cis:
                    nblk = RC_N[ci] // 32
                    for k in range(nblk):
                        lo = 128 * ci + 32 * k
                        nc.vector.tensor_copy(
                            stg[32 * k:32 * k + 3, 32 * ci:32 * ci + 32],
                            lgT[0:3, lo:lo + 32],
                        )
                    nc.vector.transpose(
                        ttok[:32 * nblk, 32 * ci:32 * ci + 32],
                        stg[:32 * nblk, 32 * ci:32 * ci + 32],
                    )

            def softmax_comb():
                """Batched softmax + top-2-of-3 + renormalize over all
                chunks; writes token-major comb into ctok."""
                t3 = ttok[:].rearrange("p (c x) -> p c x", c=NRC)[:, :, 0:3]
                p3 = prb[:].rearrange("p (c e) -> p c e", c=NRC)
                nc.scalar.activation(p3, t3, AF.Exp)
                e3 = erb.unsqueeze(1).broadcast_to([128, NRC, 3])
                nc.vector.tensor_tensor(p3, p3, e3, op=ALU.mult)
                nc.vector.tensor_reduce(ssum[:], p3, axis=AX.X, op=ALU.add)
                nc.vector.tensor_reduce(pmin[:], p3, axis=AX.X, op=ALU.min)
                nc.vector.reciprocal(rs[:], ssum[:])
                nc.vector.tensor_sub(den[:], ssum[:], pmin[:])
                nc.vector.tensor_mul(den[:], den[:], rs[:])
                nc.vector.tensor_scalar_add(den[:], den[:], 1e-6)
                nc.vector.reciprocal(invd[:], den[:])
                nc.vector.tensor_mul(t1[:], rs[:], invd[:])
                m3 = msk[:].rearrange("p (c e) -> p c e", c=NRC)
                pm3 = pmin[:].unsqueeze(2).broadcast_to([128, NRC, 3])
                nc.vector.tensor_tensor(m3, p3, pm3, op=ALU.is_gt)
                nc.vector.tensor_tensor(m3, m3, p3, op=ALU.mult)
                c3 = ctok[:].rearrange("p (c e) -> p c e", c=NRC)
                t13 = t1[:].unsqueeze(2).broadcast_to([128, NRC, 3])
                nc.vector.tensor_tensor(c3, m3, t13, op=ALU.mult)

            def load_w1g(g, nch=2):
                w1g = wp.tile([128, GH * DC * 128], F32R, tag="w1g",
                              name=f"w1g_{g}")
                step = GH * DC * 128 // nch
                for k in range(nch):
                    nc.scalar.dma_start(
                        w1g[:, k * step:(k + 1) * step],
                        w1_d[g * 128:(g + 1) * 128, k * step:(k + 1) * step])
                return w1g

            def load_w2g(g, nch=2, bsplit=False):
                w2g = wp.tile([128, GH * D], F32R, tag="w2g",
                              name=f"w2g_{g}")
                step = GH * D // nch
                for k in range(nch):
                    eng = nc.gpsimd
                    eng.dma_start(
                        w2g[:, k * step:(k + 1) * step],
                        w2_d[g * 128:(g + 1) * 128, k * step:(k + 1) * step])
                return w2g

            # deferred PE-side tasks, interleaved one per j-iteration into
            # the dense matmul stream so the PE array never cools down
            side_pe = []

            def emit_comb_transpose(ci, psH):
                n = RC_N[ci]
                tp = psH.tile([128, 512], F32, tag="h", name=f"tp_{ci}")
                nc.tensor.transpose(tp[:E, :n].bitcast(F32R),
                                    ctok[0:n, 3 * ci:3 * ci + 3],
                                    ident[:n, :n])
                nc.scalar.copy(comb_t[:, 128 * ci:128 * ci + n], tp[:E, :n])

            def emit_expand(i, psH):
                t0 = i * TN
                ex = psH.tile([128, 512], F32, tag="h", name=f"ex_{i}")
                nc.tensor.matmul(ex[:ER, :TN], bx, comb_t[:, t0:t0 + TN],
                                 start=True, stop=True)
                nc.vector.tensor_mul(scaled[:, t0:t0 + TN],
                                     acts[:, t0:t0 + TN], ex[:ER, :TN])

            # ---- main stream: 4 groups of 6 hidden chunks ----
            with (
                tc.tile_pool(name="psO", bufs=1, space="PSUM") as psO,
                tc.tile_pool(name="psH", bufs=2, space="PSUM") as psH,
            ):
                # warm the PE p-state while waiting for the xt0 DMA: dense
                # matmuls on the (already-landed) router weights; results
                # land in a junk PSUM bank that is never read
                junk = psH.tile([128, 512], F32, tag="h", name="warmps")
                for w in range(12):
                    nc.tensor.matmul(junk[:56, :280], rwd[:, 0:56],
                                     rwd[:, 0:280], start=(w == 0),
                                     stop=(w == 11))
                w1q = [load_w1g(0)]
                w2q = [load_w2g(0)]
                # late-needed consts (wu/ident/bx) after the g0 weight stream
                nc.gpsimd.dma_start(cbl[:], cbl_d[:])
                w1q.append(load_w1g(1))
                w2q.append(load_w2g(1))
                for g in range(NGROUPS):
                    if g >= 1 and g + 1 < NGROUPS:
                        w1q.append(load_w1g(g + 1))
                        w2q.append(load_w2g(g + 1))
                    w1g, w2g = w1q[g], w2q[g]

                    if g == 1:
                        # ACT is idle during g0-t3's w2 chains: do the Exp
                        # (and its two table swaps) there, then queue the 13
                        # comb transposes for interleaving into g1
                        softmax_comb()
                        side_pe.extend(
                            (lambda ci=ci: emit_comb_transpose(ci, psH))
                            for ci in range(NRC))
                    if g == 2:
                        side_pe.extend(
                            (lambda i=i: emit_expand(i, psH))
                            for i in range(NT))

                    t0 = 0
                    for nt in range(NT):
                        n = TN
                        if g == 0:
                            # merged router + LoRA-down matmul, this tile
                            dn27 = psH.tile([128, 512], F32, tag="h",
                                            name=f"dn27_{nt}")
                            for c in range(DC):
                                nc.tensor.matmul(
                                    dn27[:56, :n],
                                    rwd[:, c * 56:(c + 1) * 56],
                                    xts[nt][:, c * n:(c + 1) * n],
                                    start=(c == 0), stop=(c == DC - 1),
                                )
                            nc.scalar.copy(lgT[:, t0:t0 + n],
                                           dn27[:E, :n])
                            nc.scalar.activation(acts[:, t0:t0 + n],
                                                 dn27[32:56, :n], AF.Gelu)
                            stage_logit_chunks(CHUNKS_BY_TILE[nt])

                        outp = [psO.tile([128, 512], F32, tag=f"out{m}",
                                         name=f"out{m}_{g}_{nt}")
                                for m in range(MC)]
                        hsb = [None] * GH
                        for j in range(GH + 3):
                            if j < GH:
                                hps = psH.tile([128, 512], F32, tag="h",
                                               name=f"h_{g}_{nt}_{j}")
                                for c in range(DC):
                                    nc.tensor.matmul(
                                        hps[:, :n],
                                        w1g[:, (j * DC + c) * 128:
                                               (j * DC + c) * 128 + 128],
                                        xts[nt][:, c * n:(c + 1) * n],
                                        start=(c == 0), stop=(c == DC - 1),
                                    )
                                hsb[j] = hp.tile([128, 512], F32R, tag="hs",
                                                 name=f"hs_{g}_{nt}_{j}")
                                nc.scalar.activation(
                                    hsb[j][:, :n], hps[:, :n], AF.Gelu,
                                    bias=b1[:, g * GH + j:g * GH + j + 1],
                                )
                                if side_pe and j >= 2:
                                    side_pe.pop(0)()
                            if j >= 3:
                                jj = j - 3
                                for m in range(MC):
                                    nc.tensor.matmul(
                                        outp[m][:, :n],
                                        w2g[:, jj * D + m * 128:
                                               jj * D + m * 128 + 128],
                                        hsb[jj][:, :n],
                                        start=(jj == 0),
                                        stop=(jj == GH - 1 and g < NGROUPS - 1),
                                    )
                        if g == NGROUPS - 1:
                            for m in range(MC):
                                nc.tensor.matmul(
                                    outp[m][:, :n],
                                    wu[:, m * 128:(m + 1) * 128],
                                    scaled[:, t0:t0 + n],
                                    start=False, stop=True,
                                )
                        ob = obs[nt % 2]
                        for m in range(MC):
                            a = acc[:, m * TC + t0:m * TC + t0 + n]
                            if g == 0:
                                nc.vector.tensor_copy(a, outp[m][:, :n])
                            elif g < NGROUPS - 1:
                                nc.vector.tensor_add(a, a, outp[m][:, :n])
                            else:
                                nc.vector.scalar_tensor_tensor(
                                    ob[:, m * n:(m + 1) * n],
                                    outp[m][:, :n], b2[:, m:m + 1], a,
                                    op0=ALU.add, op1=ALU.add,
                                )
                                if m in (2, MC - 1):
                                    lo = 0 if m == 2 else 3 * n
                                    nc.sync.dma_start(
                                        out_d[nt * 128:(nt + 1) * 128,
                                              lo:(m + 1) * n],
                                        ob[:, lo:(m + 1) * n],
                                    )
                        t0 += n

    nc.compile()
    return nc


def _pack_consts(b1, b2, router_w, router_b, w_down, w_up):
    cbr = np.zeros((128, CWR), np.float32)
    rwd = np.zeros((DC, 128, 56), np.float32)
    rw = np.asarray(router_w, np.float32).reshape(DC, 128, E)
    wd = np.asarray(w_down, np.float32).transpose(1, 0, 2).reshape(DC, 128, ER)
    rwd[:, :, :E] = rw
    rwd[:, :, 32:] = wd
    cbr[:, RWD0:RWD0 + DC * 56] = rwd.transpose(1, 0, 2).reshape(128, DC * 56)
    cbl = np.zeros((128, CWL), np.float32)
    cbl[0:E, BX0:BX0 + ER] = np.repeat(np.eye(E, dtype=np.float32), R, axis=1)
    cbl[0:ER, WU0:WU0 + D] = np.asarray(w_up, np.float32).reshape(ER, D)
    cbl[:, ID0:ID0 + 128] = np.eye(128, dtype=np.float32)
    cbf = np.zeros((128, CWF), np.float32)
    cbf[:, B10:B10 + HC] = np.asarray(b1, np.float32).reshape(HC, 128).T
    cbf[:, B20:B20 + MC] = np.asarray(b2, np.float32).reshape(MC, 128).T
    cbf[:, ERB0:ERB0 + E] = np.exp(np.asarray(router_b, np.float32))[None, :]
    return cbr, cbl, cbf


def _prep_inputs(x, w1, b1, w2, b2, router_w, router_b, w_down, w_up):
    x = np.ascontiguousarray(np.asarray(x, dtype=np.float32))
    xT = x.reshape(T, D).T  # [D, T]
    # w1 [D, HID] -> [g, p, j, c, f128] -> [(g p), j*c*128]
    w1p = np.asarray(w1, np.float32).reshape(DC, 128, NGROUPS, GH, 128)
    w1p = np.ascontiguousarray(w1p.transpose(2, 1, 3, 0, 4)).reshape(
        NGROUPS * 128, GH * DC * 128)
    # w2 [HID, D] -> [g, p, j, dout] -> [(g p), j*D]
    w2p = np.asarray(w2, np.float32).reshape(NGROUPS, GH, 128, D)
    w2p = np.ascontiguousarray(w2p.transpose(0, 2, 1, 3)).reshape(
        NGROUPS * 128, GH * D)
    cbr, cbl, cbf = _pack_consts(b1, b2, router_w, router_b, w_down, w_up)
    common = {
        "w1": w1p,
        "w2": w2p,
        "cblobr": cbr,
        "cblobl": cbl,
        "cblobf": cbf,
    }
    in_maps = []
    for c in range(NCORES):
        m = dict(common)
        xc = xT[:, c * TC:(c + 1) * TC].reshape(DC, 128, NT, TN)
        m["xt"] = np.ascontiguousarray(xc.transpose(2, 1, 0, 3)).reshape(
            NT * 128, DC * TN)
        in_maps.append(m)
    return in_maps


def _run(inputs, trace=False):
    if "nc" not in _cache:
        _cache["nc"] = _build()
    nc = _cache["nc"]
    in_maps = _prep_inputs(**inputs)
    res = run_bass_kernel_spmd(nc, in_maps, core_ids=list(range(NCORES)),
                               trace=trace)
    outs = []
    for c in range(NCORES):
        a = np.asarray(res.results[c]["outT"]).astype(np.float32)
        a = a.reshape(NT, 128, MC, TN)
        outs.append(a.transpose(2, 1, 0, 3).reshape(D, TC))
    outT = np.concatenate(outs, axis=1)  # [D, T]
    out = np.ascontiguousarray(outT.T).reshape(B, N, D).astype(np.float32)
    return out, res


def kernel(**inputs):
    return _run(inputs)[0]
